# revision 11
# baseline (speedup 1.0000x reference)
"""Trainium2 Bass kernel for nn_Attention_18726057410699 (gnn_message_passing).

Math (per sample b):
  y        = local_feats[b] @ W_apair                       # [192, 256]
  binv     = binary_feats[b] @ W_bin + b_bin                # [128,128,256]
  z[i,j,k] = y[i,k] + y[j,k] + (binv[i,j,k] if i<128 and j<128 else 0)
  s[i,j]   = sigmoid( sum_k relu(z[i,j,k]) * w_att[k] + b_att )
  out[i,h] = sum_j s[i,j] * local_feats[b][j,h]

Sharding: data-parallel over batch B=8 -> 8 cores, one sample each.

Implementation notes:
 - binary_feats sent as fp8e4 flat [114, 16384] (112 channels + ones-row that
   folds b_bin + zero pad); the binv matmul runs in DoubleRow perf mode with a
   stride-0-broadcast second contraction half against a zero-padded stationary,
   so it costs 0.5 cycles/row while the DMA stays in the efficient flat layout.
 - y_i + y_j folded into PSUM by a second fp8 DoubleRow matmul whose moving
   operand is a host-sent identity+indicator pattern (per 4-row burst).
 - One relu pass moves PSUM->SBUF bf16 per [128,1024] supertile, split across
   Act and DVE (GPSIMD cannot access PSUM, so Pool is not eligible).
 - Attention scores via F=1 matmuls: stationary = relu'd z tile, moving =
   w_att column; accumulated into persistent PSUM score tiles (S^T layout).
 - Rows i>=128 (no binv term): fused add+relu from SBUF, early rows on DVE and
   late rows on Pool; symmetric block s[i<128, j>=128] filled by one PE
   transpose after an early mid-loop sigmoid.
 - Big DMAs are split across the SP/Act/Pool queues, whose transfers overlap.
"""

import numpy as np

B, N, H, L, C = 8, 192, 256, 128, 112
NIJ = L * L  # 16384

_CACHE = {}

ENGINE_SEM = {
    "EngineType.PE": "PE_",
    "EngineType.DVE": "DVE_",
    "EngineType.Activation": "Activation_",
    "EngineType.Pool": "Pool_",
    "EngineType.SP": "SP_",
}


def _fix_sync_waits(nc):
    """walrus accepts at most ONE sync-wait per compute instruction; Tile
    emits several.  Drop self-waits and push overflow onto earlier
    same-engine instructions (strictly more conservative)."""
    import dataclasses
    from collections import defaultdict

    il = [i for i in nc.all_instructions()]
    streams = defaultdict(list)
    for inst in il:
        si = getattr(inst, "sync_info", None)
        if si is None:
            continue
        upd = {u.ant_name for u in si.on_update}
        eng = str(getattr(inst, "engine", None))
        self_pfx = ENGINE_SEM.get(eng)
        keep = {}
        for w in si.on_wait:
            if w.ant_name in upd:
                continue
            if self_pfx and w.ant_name.startswith(self_pfx):
                continue
            k = w.ant_name
            if k not in keep or keep[k].wait_value < w.wait_value:
                keep[k] = w
        new = list(keep.values())
        if len(new) != len(si.on_wait):
            inst.sync_info = dataclasses.replace(si, on_wait=new)
        if type(inst).__name__ in (
            "InstMatmult", "InstTensorCopy", "InstTensorTensor",
            "InstTensorScalarPtr", "InstActivation", "InstMemset",
            "InstTensorReduce", "InstTensorTensorReduce",
        ):
            streams[eng].append(inst)

    for eng, insts in streams.items():
        overflow = []
        for inst in reversed(insts):
            si = inst.sync_info
            waits = list(si.on_wait) + overflow
            ded = {}
            for w in waits:
                if w.ant_name not in ded or ded[w.ant_name].wait_value < w.wait_value:
                    ded[w.ant_name] = w
            waits = list(ded.values())
            if len(waits) <= 1:
                inst.sync_info = dataclasses.replace(si, on_wait=waits)
                overflow = []
            else:
                inst.sync_info = dataclasses.replace(si, on_wait=[waits[-1]])
                overflow = waits[:-1]
        if overflow:
            raise RuntimeError(f"{eng}: could not place {len(overflow)} waits")


HI_SPLIT = 12  # region-B rows below this go to DVE, the rest to Pool

# relu-move engine schedule per supertile (32 supertiles = 32 bursts x 2kt):
# legal engines for PSUM reads: Act + DVE only
RELU_ENG = (
    "A A A D A A D A D A D A D A D A "
    "D A D A D A D A D A D A D A D A"
).split()


def _build():
    import concourse.bass as bass
    import concourse.tile as tile
    from concourse import bacc, mybir

    f32 = mybir.dt.float32
    bf16 = mybir.dt.bfloat16
    fp8 = mybir.dt.float8e4
    ALU = mybir.AluOpType
    ACTF = mybir.ActivationFunctionType
    PM = mybir.MatmulPerfMode

    nc = bacc.Bacc()

    p_bin = nc.declare_dram_parameter("bin8", [114, NIJ], fp8, isOutput=False)
    p_mv2 = nc.declare_dram_parameter("mv28", [128, 32, 512], fp8, isOutput=False)
    p_wbin = nc.declare_dram_parameter("wbin8", [114, 2, H], fp8, isOutput=False)
    # packed bf16 smalls: [xw (2*(N+H)) | xb (H) | xb1-pad (H) | watt (2) | eye (128)]
    PK = 2 * (N + H) + H + H + 2 + 128
    p_pk = nc.declare_dram_parameter("pk", [128, PK], bf16, isOutput=False)
    p_batt = nc.declare_dram_parameter("battc", [128, 1], f32, isOutput=False)
    p_out = nc.declare_dram_parameter("out", [N, H], f32, isOutput=True)

    with tile.TileContext(nc) as tc:
        with (
            tc.tile_pool(name="const", bufs=1) as cpool,
            tc.tile_pool(name="work", bufs=4) as wpool,
            tc.tile_pool(name="ub", bufs=66) as ubpool,
            tc.tile_pool(name="fin", bufs=1) as fpool,
            tc.tile_pool(name="pmain", bufs=3, space=bass.MemorySpace.PSUM) as pmain,
            tc.tile_pool(name="pscore", bufs=1, space=bass.MemorySpace.PSUM) as psc,
        ):
            # ---------------- DMA plan (flat 2D layouts; queues overlap) ----------------
            # SP: pk, battc, mv28 q3 q4, outs
            # Act-queue: bin8-h1 (bursts 0-15); Pool-queue: mv28 q1 q2, bin8-h2, wbin8
            bin8 = cpool.tile([114, NIJ], fp8, tag="bin8")
            mv28 = cpool.tile([128, 32, 512], fp8, tag="mv28")
            yn08 = cpool.tile([128, 2, H], fp8, tag="yn08")
            actwarm = cpool.tile([1, 2], bf16, tag="actwarm")
            nc.gpsimd.memset(actwarm[:, :], 0.0)
            nc.gpsimd.memset(yn08[:, 1, :], 0.0)
            # warm the sigmoid table at t=0 (its func-set also contains Relu)
            nc.scalar.activation(actwarm[:, 1:2], actwarm[:, 0:1], ACTF.Sigmoid, bias=0.0, scale=1.0)
            HLF = NIJ // 2
            QTR = 8
            pk = cpool.tile([128, PK], bf16, tag="pk")
            nc.sync.dma_start(pk[:, :], p_pk[:, :])
            wbin8 = cpool.tile([114, 2, H], fp8, tag="wbin8")
            nc.sync.dma_start(wbin8[:, :, :], p_wbin[:, :, :])
            bc_sb = cpool.tile([128, 1], f32, tag="battc")
            BQ = NIJ // 4
            BE = NIJ // 8
            nc.sync.dma_start(bin8[:, 0:BE], p_bin[:, 0:BE])
            nc.gpsimd.dma_start(mv28[:, 0:QTR, :], p_mv2[:, 0:QTR, :])
            nc.sync.dma_start(bin8[:, BE:BQ], p_bin[:, BE:BQ])
            nc.sync.dma_start(bin8[:, BQ : 2 * BQ], p_bin[:, BQ : 2 * BQ])
            nc.gpsimd.dma_start(mv28[:, QTR : 2 * QTR, :], p_mv2[:, QTR : 2 * QTR, :])
            nc.sync.dma_start(bin8[:, 2 * BQ : 3 * BQ], p_bin[:, 2 * BQ : 3 * BQ])
            nc.sync.dma_start(bin8[:, 3 * BQ : 4 * BQ], p_bin[:, 3 * BQ : 4 * BQ])
            nc.sync.dma_start(mv28[:, 2 * QTR : 3 * QTR, :], p_mv2[:, 2 * QTR : 3 * QTR, :])
            nc.sync.dma_start(mv28[:, 3 * QTR : 4 * QTR, :], p_mv2[:, 3 * QTR : 4 * QTR, :])
            nc.sync.dma_start(bc_sb[:, :], p_batt[:, :])

            o_xw = 0
            o_xb = 2 * (N + H)
            o_xb1 = o_xb + H
            o_wa = o_xb1 + H
            o_eye = o_wa + 2
            xw = pk[:, o_xw : o_xw + 2 * (N + H)].rearrange("p (t n) -> p t n", t=2)
            wa = pk[:, o_wa : o_wa + 2]
            xb = pk[:, o_xb : o_xb + H]
            xb1 = pk[0:64, o_xb1 : o_xb1 + H]
            eye_b = pk[:, o_eye : o_eye + 128]

            # ---------------- y matmuls (bf16) ----------------
            # Yn (fold stationary) first: it gates the first bursts
            ynp = pmain.tile([128, 2, 512], f32, tag="super")
            for ht in range(2):
                nc.tensor.matmul(
                    ynp[:, 0, 0:H],
                    xw[:, ht, 0:128],
                    xw[:, ht, N : N + H],
                    start=(ht == 0),
                    stop=(ht == 1),
                )
            nc.scalar.copy(yn08[:, 0, :], ynp[:, 0, 0:H])

            ytp = []
            for kt in range(2):
                ps = pmain.tile([128, 2, 512], f32, tag="super")
                for ht in range(2):
                    nc.tensor.matmul(
                        ps[:, 0, 0:N],
                        xw[:, ht, N + kt * 128 : N + (kt + 1) * 128],
                        xw[:, ht, 0:N],
                        start=(ht == 0),
                        stop=(ht == 1),
                    )
                ytp.append(ps)
            ytb = cpool.tile([128, 2, N], bf16, tag="ytb")
            yt_f = cpool.tile([128, 2, N], f32, tag="ytf")
            for kt in range(2):
                nc.vector.tensor_copy(ytb[:, kt, :], ytp[kt][:, 0, 0:N])
                nc.scalar.copy(yt_f[:, kt, :], ytp[kt][:, 0, 0:N])

            # ---------------- score PSUM tiles (persistent) ----------------
            stt = psc.tile([128, 2, N], f32, tag="st")
            st0 = stt[:, 0, :]                            # S^T[j<128, i]
            st1 = stt[0:64, 1, :]                         # S^T[j>=128, i] (cols i>=128 only)

            # ---------------- main interleaved loop ----------------
            hi_tiles = {}
            lo_tiles = {}

            def hi_compute(t):
                # region B: i = 128 + t; early rows on DVE, late rows on Pool
                # (GPSIMD cannot touch PSUM, so Pool earns its keep here)
                i = 128 + t
                u = ubpool.tile([128, 2, N], bf16, tag="ub")
                eng = nc.vector if t < HI_SPLIT else nc.gpsimd
                for kt in range(2):
                    eng.tensor_scalar(
                        u[:, kt, :], ytb[:, kt, :],
                        yt_f[:, kt, i : i + 1], 0.0,
                        ALU.add, ALU.max,
                    )
                hi_tiles[t] = u

            def hi_scores(t):
                i = 128 + t
                u = hi_tiles.pop(t)
                for kt in range(2):
                    nc.tensor.matmul(
                        stt[:, 0, i : i + 1], u[:, kt, 0:128], wa[:, kt : kt + 1],
                        start=(kt == 0), stop=(kt == 1), skip_group_check=True,
                    )
                for kt in range(2):
                    nc.tensor.matmul(
                        stt[0:64, 1, i : i + 1], u[:, kt, 128:N], wa[:, kt : kt + 1],
                        start=(kt == 0), stop=(kt == 1), skip_group_check=True,
                    )

            def lo_burst(ib):
                # region A: i = 4*ib..4*ib+3, j<128; z for both kt in one supertile
                ps = pmain.tile([128, 2, 512], f32, tag="super")
                rhs_bin = bin8[:, ib * 512 : (ib + 1) * 512].unsqueeze(1).broadcast_to([114, 2, 512])
                rhs_mv = mv28[:, ib, :].unsqueeze(1).broadcast_to([128, 2, 512])
                for kt in range(2):
                    nc.tensor.matmul(
                        ps[:, kt, :],
                        wbin8[:, :, kt * 128 : (kt + 1) * 128],
                        rhs_bin,
                        start=True, stop=False,
                        perf_mode=PM.DoubleRow, skip_group_check=True,
                    )
                    nc.tensor.matmul(
                        ps[:, kt, :],
                        yn08[:, :, kt * 128 : (kt + 1) * 128],
                        rhs_mv,
                        start=False, stop=True,
                        perf_mode=PM.DoubleRow, skip_group_check=True,
                    )
                u = wpool.tile([128, 2, 4, 128], bf16, tag="u")
                if RELU_ENG[ib] == "A":
                    nc.scalar.activation(u[:, :, :, :], ps[:, :, :], ACTF.Relu, bias=0.0, scale=1.0)
                else:
                    nc.vector.tensor_scalar(u[:, :, :, :], ps[:, :, :], 0.0, None, ALU.max)
                lo_tiles[ib] = u

            def lo_scores(ib):
                u = lo_tiles.pop(ib)
                for m in range(4):
                    i = 4 * ib + m
                    for kt in range(2):
                        nc.tensor.matmul(
                            stt[:, 0, i : i + 1], u[:, kt, m, :], wa[:, kt : kt + 1],
                            start=(kt == 0), stop=(kt == 1), skip_group_check=True,
                        )

            # all region-B DVE work upfront (independent of binT DMA);
            # their cheap F=1 score matmuls interleave between bursts
            for t in range(64):
                hi_compute(t)
            hi_next = 0
            ss0 = fpool.tile([128, N], bf16, tag="ss0")
            ss1 = fpool.tile([64, N], bf16, tag="ss1")
            for ib in range(32):
                lo_burst(ib)
                if ib >= 2:
                    lo_scores(ib - 2)
                nhi = 2 if ib < 20 else 3
                for _ in range(nhi):
                    if hi_next < 64:
                        hi_scores(hi_next)
                        hi_next += 1
                if ib == 30:
                    # B-columns complete: sigmoid them, mirror, and do the
                    # i>=128 half of the finale while region A still runs
                    nc.scalar.activation(ss0[:, 128:N], stt[:, 0, 128:N], ACTF.Sigmoid,
                                         bias=bc_sb[:, 0:1], scale=1.0)
                    nc.scalar.activation(ss1[:, 128:N], stt[0:64, 1, 128:N], ACTF.Sigmoid,
                                         bias=bc_sb[0:64, 0:1], scale=1.0)
                    pst = psc.tile([64, 128], bf16, tag="trsp")
                    nc.tensor.transpose(pst[:, :], ss0[:, 128:N], eye_b)
                    nc.vector.tensor_copy(ss1[:, 0:128], pst[:, :])

            lo_scores(30)
            lo_scores(31)

            # ---------------- finale ----------------
            # i>=128 half first (B-side sigmoids already done mid-loop)
            po1 = pmain.tile([128, 2, 512], f32, tag="super")
            nc.tensor.matmul(po1[0:64, 0, 0:H], ss0[:, 128:N], xb, start=True, stop=False)
            nc.tensor.matmul(po1[0:64, 0, 0:H], ss1[:, 128:N], xb1, start=False, stop=True)
            nc.scalar.activation(ss0[:, 0:128], stt[:, 0, 0:128], ACTF.Sigmoid,
                                 bias=bc_sb[:, 0:1], scale=1.0)
            ob1 = fpool.tile([64, H], f32, tag="ob1")
            nc.vector.tensor_copy(ob1[:, :], po1[0:64, 0, 0:H])
            nc.sync.dma_start(p_out[128:N, :], ob1[:, :])
            po = pmain.tile([128, 2, 512], f32, tag="super")
            nc.tensor.matmul(po[:, 0, 0:H], ss1[:, 0:128], xb1, start=True, stop=False)
            nc.tensor.matmul(po[:, 0, 0:H], ss0[:, 0:128], xb, start=False, stop=True)
            ob = fpool.tile([128, H], f32, tag="ob0")
            nc.vector.tensor_copy(ob[:, :], po[:, 0, 0:H])
            nc.sync.dma_start(p_out[0:128, :], ob[:, :])

    nc.compile()
    return nc


def _mv2_const():
    import ml_dtypes
    # mv2[p, ib, m*128+j] = I[p == j] + I[p == 4*ib+m]
    mv = np.zeros((128, 32, 4, 128), np.float32)
    for p in range(128):
        mv[p, :, :, p] += 1.0
        ib, m = divmod(p, 4)
        mv[p, ib, m, :] += 1.0
    return np.ascontiguousarray(mv.reshape(128, 32, 512)).astype(ml_dtypes.float8_e4m3)


def _prep_inputs(local_feats, binary_feats, W_apair, W_bin, b_bin, w_att, b_att):
    import ml_dtypes
    fp8 = ml_dtypes.float8_e4m3

    lf = np.asarray(local_feats, np.float32)
    bf = np.asarray(binary_feats, np.float32)
    wap = np.ascontiguousarray(np.asarray(W_apair, np.float32))
    wbin = np.asarray(W_bin, np.float32)
    bb = np.asarray(b_bin, np.float32).reshape(H)
    wa = np.asarray(w_att, np.float32).reshape(H)
    battc = np.full((128, 1), np.float32(np.asarray(b_att).reshape(-1)[0]), np.float32)

    # wbin8[p, d, k]: d0 = wbin_aug row p (c=112 -> b_bin, c=113 -> 0), d1 = zeros
    wbin_aug = np.zeros((114, 2, H), np.float32)
    wbin_aug[:C, 0] = wbin
    wbin_aug[C, 0] = bb
    wbin8 = np.ascontiguousarray(wbin_aug).astype(fp8)

    mv28 = _CACHE.get("mv28")
    if mv28 is None:
        mv28 = _CACHE["mv28"] = _mv2_const()
    watt = np.ascontiguousarray(wa.reshape(2, 128).T)
    eye = np.eye(128, dtype=np.float32)

    in_maps = []
    for b in range(B):
        # bin8[c, ij]: rows 0..111 = binT, 112 = ones (b_bin), 113 = zeros
        binT = bf[b].reshape(NIJ, C).T
        bin_aug = np.empty((114, NIJ), np.float32)
        bin_aug[:C] = binT
        bin_aug[C] = 1.0
        bin_aug[C + 1] = 0.0
        bin8 = np.ascontiguousarray(bin_aug).astype(fp8)

        # xw[p, ht, :]: cols 0..191 = x^T[h, j] (h = ht*128+p), cols 192.. = W_apair[h, k]
        xT = lf[b].T  # [256, 192]
        xw = np.concatenate(
            [xT.reshape(2, 128, N), wap.reshape(2, 128, H)], axis=2
        ).transpose(1, 0, 2)  # [128, 2, N+H]
        xb1p = np.zeros((128, H), np.float32)
        xb1p[:64] = lf[b][128:]
        pk = np.concatenate(
            [
                xw.reshape(128, 2 * (N + H)),
                lf[b][:128],
                xb1p,
                watt,
                eye,
            ],
            axis=1,
        )
        in_maps.append(
            {
                "bin8": bin8,
                "mv28": mv28,
                "wbin8": wbin8,
                "pk": np.ascontiguousarray(pk).astype(ml_dtypes.bfloat16),
                "battc": battc,
            }
        )
    return in_maps


def run_full(inputs, trace=False):
    from concourse.bass_utils import run_bass_kernel_spmd

    if "nc" not in _CACHE:
        _CACHE["nc"] = _build()
    nc = _CACHE["nc"]
    in_maps = _prep_inputs(
        inputs["local_feats"],
        inputs["binary_feats"],
        inputs["W_apair"],
        inputs["W_bin"],
        inputs["b_bin"],
        inputs["w_att"],
        inputs["b_att"],
    )
    res = run_bass_kernel_spmd(nc, in_maps, list(range(B)), trace=trace)
    out = np.stack([np.asarray(res.results[c]["out"], np.float32) for c in range(B)])
    return out, res


def kernel(**inputs):
    out, _ = run_full(inputs, trace=False)
    return out


# revision 12
# speedup vs baseline: 1.0024x; 1.0024x over previous
"""Trainium2 Bass kernel for nn_Attention_18726057410699 (gnn_message_passing).

Math (per sample b):
  y        = local_feats[b] @ W_apair                       # [192, 256]
  binv     = binary_feats[b] @ W_bin + b_bin                # [128,128,256]
  z[i,j,k] = y[i,k] + y[j,k] + (binv[i,j,k] if i<128 and j<128 else 0)
  s[i,j]   = sigmoid( sum_k relu(z[i,j,k]) * w_att[k] + b_att )
  out[i,h] = sum_j s[i,j] * local_feats[b][j,h]

Sharding: data-parallel over batch B=8 -> 8 cores, one sample each.

Implementation notes:
 - binary_feats sent as fp8e4 flat [114, 16384] (112 channels + ones-row that
   folds b_bin + zero pad); the binv matmul runs in DoubleRow perf mode with a
   stride-0-broadcast second contraction half against a zero-padded stationary,
   so it costs 0.5 cycles/row while the DMA stays in the efficient flat layout.
 - y_i + y_j folded into PSUM by a second fp8 DoubleRow matmul whose moving
   operand is a host-sent identity+indicator pattern (per 4-row burst).
 - One relu pass moves PSUM->SBUF bf16 per [128,1024] supertile, split across
   Act and DVE (GPSIMD cannot access PSUM, so Pool is not eligible).
 - Attention scores via F=1 matmuls: stationary = relu'd z tile, moving =
   w_att column; accumulated into persistent PSUM score tiles (S^T layout).
 - Rows i>=128 (no binv term): fused add+relu from SBUF, early rows on DVE and
   late rows on Pool; symmetric block s[i<128, j>=128] filled by one PE
   transpose after an early mid-loop sigmoid.
 - Big DMAs are split across the SP/Act/Pool queues, whose transfers overlap.
"""

import numpy as np

B, N, H, L, C = 8, 192, 256, 128, 112
NIJ = L * L  # 16384

_CACHE = {}

ENGINE_SEM = {
    "EngineType.PE": "PE_",
    "EngineType.DVE": "DVE_",
    "EngineType.Activation": "Activation_",
    "EngineType.Pool": "Pool_",
    "EngineType.SP": "SP_",
}


def _fix_sync_waits(nc):
    """walrus accepts at most ONE sync-wait per compute instruction; Tile
    emits several.  Drop self-waits and push overflow onto earlier
    same-engine instructions (strictly more conservative)."""
    import dataclasses
    from collections import defaultdict

    il = [i for i in nc.all_instructions()]
    streams = defaultdict(list)
    for inst in il:
        si = getattr(inst, "sync_info", None)
        if si is None:
            continue
        upd = {u.ant_name for u in si.on_update}
        eng = str(getattr(inst, "engine", None))
        self_pfx = ENGINE_SEM.get(eng)
        keep = {}
        for w in si.on_wait:
            if w.ant_name in upd:
                continue
            if self_pfx and w.ant_name.startswith(self_pfx):
                continue
            k = w.ant_name
            if k not in keep or keep[k].wait_value < w.wait_value:
                keep[k] = w
        new = list(keep.values())
        if len(new) != len(si.on_wait):
            inst.sync_info = dataclasses.replace(si, on_wait=new)
        if type(inst).__name__ in (
            "InstMatmult", "InstTensorCopy", "InstTensorTensor",
            "InstTensorScalarPtr", "InstActivation", "InstMemset",
            "InstTensorReduce", "InstTensorTensorReduce",
        ):
            streams[eng].append(inst)

    for eng, insts in streams.items():
        overflow = []
        for inst in reversed(insts):
            si = inst.sync_info
            waits = list(si.on_wait) + overflow
            ded = {}
            for w in waits:
                if w.ant_name not in ded or ded[w.ant_name].wait_value < w.wait_value:
                    ded[w.ant_name] = w
            waits = list(ded.values())
            if len(waits) <= 1:
                inst.sync_info = dataclasses.replace(si, on_wait=waits)
                overflow = []
            else:
                inst.sync_info = dataclasses.replace(si, on_wait=[waits[-1]])
                overflow = waits[:-1]
        if overflow:
            raise RuntimeError(f"{eng}: could not place {len(overflow)} waits")


HI_SPLIT = 12  # region-B rows below this go to DVE, the rest to Pool

# relu-move engine schedule per supertile (32 supertiles = 32 bursts x 2kt):
# legal engines for PSUM reads: Act + DVE only
RELU_ENG = (
    "A A A D A A D A D A D A D A D A "
    "D A D A D A D A D A D A D A D A"
).split()


def _build():
    import concourse.bass as bass
    import concourse.tile as tile
    from concourse import bacc, mybir

    f32 = mybir.dt.float32
    bf16 = mybir.dt.bfloat16
    fp8 = mybir.dt.float8e4
    ALU = mybir.AluOpType
    ACTF = mybir.ActivationFunctionType
    PM = mybir.MatmulPerfMode

    nc = bacc.Bacc()

    p_bin = nc.declare_dram_parameter("bin8", [114, NIJ], fp8, isOutput=False)
    p_mv2 = nc.declare_dram_parameter("mv28", [128, 32, 512], fp8, isOutput=False)
    p_wbin = nc.declare_dram_parameter("wbin8", [114, 2, H], fp8, isOutput=False)
    # packed bf16 smalls: [xw (2*(N+H)) | xb (H) | xb1-pad (H) | watt (2) | eye (128)]
    PK = 2 * (N + H) + H + H + 2 + 128
    p_pk = nc.declare_dram_parameter("pk", [128, PK], bf16, isOutput=False)
    p_batt = nc.declare_dram_parameter("battc", [128, 1], f32, isOutput=False)
    p_out = nc.declare_dram_parameter("out", [N, H], f32, isOutput=True)

    with tile.TileContext(nc) as tc:
        with (
            tc.tile_pool(name="const", bufs=1) as cpool,
            tc.tile_pool(name="work", bufs=4) as wpool,
            tc.tile_pool(name="ub", bufs=66) as ubpool,
            tc.tile_pool(name="fin", bufs=1) as fpool,
            tc.tile_pool(name="pmain", bufs=3, space=bass.MemorySpace.PSUM) as pmain,
            tc.tile_pool(name="pscore", bufs=1, space=bass.MemorySpace.PSUM) as psc,
        ):
            # ---------------- DMA plan (flat 2D layouts; queues overlap) ----------------
            # SP: pk, battc, mv28 q3 q4, outs
            # Act-queue: bin8-h1 (bursts 0-15); Pool-queue: mv28 q1 q2, bin8-h2, wbin8
            bin8 = cpool.tile([114, NIJ], fp8, tag="bin8")
            mv28 = cpool.tile([128, 32, 512], fp8, tag="mv28")
            yn08 = cpool.tile([128, 2, H], fp8, tag="yn08")
            actwarm = cpool.tile([1, 2], bf16, tag="actwarm")
            nc.gpsimd.memset(actwarm[:, :], 0.0)
            nc.gpsimd.memset(yn08[:, 1, :], 0.0)
            # warm the sigmoid table at t=0 (its func-set also contains Relu)
            nc.scalar.activation(actwarm[:, 1:2], actwarm[:, 0:1], ACTF.Sigmoid, bias=0.0, scale=1.0)
            HLF = NIJ // 2
            QTR = 8
            pk = cpool.tile([128, PK], bf16, tag="pk")
            nc.sync.dma_start(pk[:, :], p_pk[:, :])
            wbin8 = cpool.tile([114, 2, H], fp8, tag="wbin8")
            nc.sync.dma_start(wbin8[:, :, :], p_wbin[:, :, :])
            bc_sb = cpool.tile([128, 1], f32, tag="battc")
            BQ = NIJ // 4
            BE = NIJ // 8
            nc.sync.dma_start(bin8[:, 0:BE], p_bin[:, 0:BE])
            nc.gpsimd.dma_start(mv28[:, 0:QTR, :], p_mv2[:, 0:QTR, :])
            nc.sync.dma_start(bin8[:, BE:BQ], p_bin[:, BE:BQ])
            nc.sync.dma_start(bin8[:, BQ : 2 * BQ], p_bin[:, BQ : 2 * BQ])
            nc.gpsimd.dma_start(mv28[:, QTR : 2 * QTR, :], p_mv2[:, QTR : 2 * QTR, :])
            nc.sync.dma_start(bin8[:, 2 * BQ : 3 * BQ], p_bin[:, 2 * BQ : 3 * BQ])
            nc.sync.dma_start(bin8[:, 3 * BQ : 4 * BQ], p_bin[:, 3 * BQ : 4 * BQ])
            nc.sync.dma_start(mv28[:, 2 * QTR : 3 * QTR, :], p_mv2[:, 2 * QTR : 3 * QTR, :])
            nc.sync.dma_start(mv28[:, 3 * QTR : 4 * QTR, :], p_mv2[:, 3 * QTR : 4 * QTR, :])
            nc.sync.dma_start(bc_sb[:, :], p_batt[:, :])

            o_xw = 0
            o_xb = 2 * (N + H)
            o_xb1 = o_xb + H
            o_wa = o_xb1 + H
            o_eye = o_wa + 2
            xw = pk[:, o_xw : o_xw + 2 * (N + H)].rearrange("p (t n) -> p t n", t=2)
            wa = pk[:, o_wa : o_wa + 2]
            xb = pk[:, o_xb : o_xb + H]
            xb1 = pk[0:64, o_xb1 : o_xb1 + H]
            eye_b = pk[:, o_eye : o_eye + 128]

            # ---------------- y matmuls (bf16) ----------------
            # Yn (fold stationary) first: it gates the first bursts
            ynp = pmain.tile([128, 2, 512], f32, tag="super")
            for ht in range(2):
                nc.tensor.matmul(
                    ynp[:, 0, 0:H],
                    xw[:, ht, 0:128],
                    xw[:, ht, N : N + H],
                    start=(ht == 0),
                    stop=(ht == 1),
                )
            nc.scalar.copy(yn08[:, 0, :], ynp[:, 0, 0:H])

            ytp = []
            for kt in range(2):
                ps = pmain.tile([128, 2, 512], f32, tag="super")
                for ht in range(2):
                    nc.tensor.matmul(
                        ps[:, 0, 0:N],
                        xw[:, ht, N + kt * 128 : N + (kt + 1) * 128],
                        xw[:, ht, 0:N],
                        start=(ht == 0),
                        stop=(ht == 1),
                    )
                ytp.append(ps)
            ytb = cpool.tile([128, 2, N], bf16, tag="ytb")
            yt_f = cpool.tile([128, 2, N], f32, tag="ytf")
            for kt in range(2):
                nc.vector.tensor_copy(ytb[:, kt, :], ytp[kt][:, 0, 0:N])
                nc.vector.tensor_copy(yt_f[:, kt, :], ytp[kt][:, 0, 0:N])

            # ---------------- score PSUM tiles (persistent) ----------------
            stt = psc.tile([128, 2, N], f32, tag="st")
            st0 = stt[:, 0, :]                            # S^T[j<128, i]
            st1 = stt[0:64, 1, :]                         # S^T[j>=128, i] (cols i>=128 only)

            # ---------------- main interleaved loop ----------------
            hi_tiles = {}
            lo_tiles = {}

            def hi_compute(t):
                # region B: i = 128 + t; early rows on DVE, late rows on Pool
                # (GPSIMD cannot touch PSUM, so Pool earns its keep here)
                i = 128 + t
                u = ubpool.tile([128, 2, N], bf16, tag="ub")
                eng = nc.vector if t < HI_SPLIT else nc.gpsimd
                for kt in range(2):
                    eng.tensor_scalar(
                        u[:, kt, :], ytb[:, kt, :],
                        yt_f[:, kt, i : i + 1], 0.0,
                        ALU.add, ALU.max,
                    )
                hi_tiles[t] = u

            def hi_scores(t):
                i = 128 + t
                u = hi_tiles.pop(t)
                for kt in range(2):
                    nc.tensor.matmul(
                        stt[:, 0, i : i + 1], u[:, kt, 0:128], wa[:, kt : kt + 1],
                        start=(kt == 0), stop=(kt == 1), skip_group_check=True,
                    )
                for kt in range(2):
                    nc.tensor.matmul(
                        stt[0:64, 1, i : i + 1], u[:, kt, 128:N], wa[:, kt : kt + 1],
                        start=(kt == 0), stop=(kt == 1), skip_group_check=True,
                    )

            def lo_burst(ib):
                # region A: i = 4*ib..4*ib+3, j<128; z for both kt in one supertile
                ps = pmain.tile([128, 2, 512], f32, tag="super")
                rhs_bin = bin8[:, ib * 512 : (ib + 1) * 512].unsqueeze(1).broadcast_to([114, 2, 512])
                rhs_mv = mv28[:, ib, :].unsqueeze(1).broadcast_to([128, 2, 512])
                for kt in range(2):
                    nc.tensor.matmul(
                        ps[:, kt, :],
                        wbin8[:, :, kt * 128 : (kt + 1) * 128],
                        rhs_bin,
                        start=True, stop=False,
                        perf_mode=PM.DoubleRow, skip_group_check=True,
                    )
                    nc.tensor.matmul(
                        ps[:, kt, :],
                        yn08[:, :, kt * 128 : (kt + 1) * 128],
                        rhs_mv,
                        start=False, stop=True,
                        perf_mode=PM.DoubleRow, skip_group_check=True,
                    )
                u = wpool.tile([128, 2, 4, 128], bf16, tag="u")
                if RELU_ENG[ib] == "A":
                    nc.scalar.activation(u[:, :, :, :], ps[:, :, :], ACTF.Relu, bias=0.0, scale=1.0)
                else:
                    nc.vector.tensor_scalar(u[:, :, :, :], ps[:, :, :], 0.0, None, ALU.max)
                lo_tiles[ib] = u

            def lo_scores(ib):
                u = lo_tiles.pop(ib)
                for m in range(4):
                    i = 4 * ib + m
                    for kt in range(2):
                        nc.tensor.matmul(
                            stt[:, 0, i : i + 1], u[:, kt, m, :], wa[:, kt : kt + 1],
                            start=(kt == 0), stop=(kt == 1), skip_group_check=True,
                        )

            # all region-B DVE work upfront (independent of binT DMA);
            # their cheap F=1 score matmuls interleave between bursts
            for t in range(64):
                hi_compute(t)
            hi_next = 0
            ss0 = fpool.tile([128, N], bf16, tag="ss0")
            ss1 = fpool.tile([64, N], bf16, tag="ss1")
            for ib in range(32):
                lo_burst(ib)
                if ib >= 2:
                    lo_scores(ib - 2)
                nhi = 2 if ib < 20 else 3
                for _ in range(nhi):
                    if hi_next < 64:
                        hi_scores(hi_next)
                        hi_next += 1
                if ib == 30:
                    # B-columns complete: sigmoid them, mirror, and do the
                    # i>=128 half of the finale while region A still runs
                    nc.scalar.activation(ss0[:, 128:N], stt[:, 0, 128:N], ACTF.Sigmoid,
                                         bias=bc_sb[:, 0:1], scale=1.0)
                    nc.scalar.activation(ss1[:, 128:N], stt[0:64, 1, 128:N], ACTF.Sigmoid,
                                         bias=bc_sb[0:64, 0:1], scale=1.0)
                    pst = psc.tile([64, 128], bf16, tag="trsp")
                    nc.tensor.transpose(pst[:, :], ss0[:, 128:N], eye_b)
                    nc.vector.tensor_copy(ss1[:, 0:128], pst[:, :])

            lo_scores(30)
            lo_scores(31)

            # ---------------- finale ----------------
            # i>=128 half first (B-side sigmoids already done mid-loop)
            po1 = pmain.tile([128, 2, 512], f32, tag="super")
            nc.tensor.matmul(po1[0:64, 0, 0:H], ss0[:, 128:N], xb, start=True, stop=False)
            nc.tensor.matmul(po1[0:64, 0, 0:H], ss1[:, 128:N], xb1, start=False, stop=True)
            nc.scalar.activation(ss0[:, 0:128], stt[:, 0, 0:128], ACTF.Sigmoid,
                                 bias=bc_sb[:, 0:1], scale=1.0)
            ob1 = fpool.tile([64, H], f32, tag="ob1")
            nc.vector.tensor_copy(ob1[:, :], po1[0:64, 0, 0:H])
            nc.sync.dma_start(p_out[128:N, :], ob1[:, :])
            po = pmain.tile([128, 2, 512], f32, tag="super")
            nc.tensor.matmul(po[:, 0, 0:H], ss1[:, 0:128], xb1, start=True, stop=False)
            nc.tensor.matmul(po[:, 0, 0:H], ss0[:, 0:128], xb, start=False, stop=True)
            ob = fpool.tile([128, H], f32, tag="ob0")
            nc.vector.tensor_copy(ob[:, :], po[:, 0, 0:H])
            nc.sync.dma_start(p_out[0:128, :], ob[:, :])

    nc.compile()
    return nc


def _mv2_const():
    import ml_dtypes
    # mv2[p, ib, m*128+j] = I[p == j] + I[p == 4*ib+m]
    mv = np.zeros((128, 32, 4, 128), np.float32)
    for p in range(128):
        mv[p, :, :, p] += 1.0
        ib, m = divmod(p, 4)
        mv[p, ib, m, :] += 1.0
    return np.ascontiguousarray(mv.reshape(128, 32, 512)).astype(ml_dtypes.float8_e4m3)


def _prep_inputs(local_feats, binary_feats, W_apair, W_bin, b_bin, w_att, b_att):
    import ml_dtypes
    fp8 = ml_dtypes.float8_e4m3

    lf = np.asarray(local_feats, np.float32)
    bf = np.asarray(binary_feats, np.float32)
    wap = np.ascontiguousarray(np.asarray(W_apair, np.float32))
    wbin = np.asarray(W_bin, np.float32)
    bb = np.asarray(b_bin, np.float32).reshape(H)
    wa = np.asarray(w_att, np.float32).reshape(H)
    battc = np.full((128, 1), np.float32(np.asarray(b_att).reshape(-1)[0]), np.float32)

    # wbin8[p, d, k]: d0 = wbin_aug row p (c=112 -> b_bin, c=113 -> 0), d1 = zeros
    wbin_aug = np.zeros((114, 2, H), np.float32)
    wbin_aug[:C, 0] = wbin
    wbin_aug[C, 0] = bb
    wbin8 = np.ascontiguousarray(wbin_aug).astype(fp8)

    mv28 = _CACHE.get("mv28")
    if mv28 is None:
        mv28 = _CACHE["mv28"] = _mv2_const()
    watt = np.ascontiguousarray(wa.reshape(2, 128).T)
    eye = np.eye(128, dtype=np.float32)

    in_maps = []
    for b in range(B):
        # bin8[c, ij]: rows 0..111 = binT, 112 = ones (b_bin), 113 = zeros
        binT = bf[b].reshape(NIJ, C).T
        bin_aug = np.empty((114, NIJ), np.float32)
        bin_aug[:C] = binT
        bin_aug[C] = 1.0
        bin_aug[C + 1] = 0.0
        bin8 = np.ascontiguousarray(bin_aug).astype(fp8)

        # xw[p, ht, :]: cols 0..191 = x^T[h, j] (h = ht*128+p), cols 192.. = W_apair[h, k]
        xT = lf[b].T  # [256, 192]
        xw = np.concatenate(
            [xT.reshape(2, 128, N), wap.reshape(2, 128, H)], axis=2
        ).transpose(1, 0, 2)  # [128, 2, N+H]
        xb1p = np.zeros((128, H), np.float32)
        xb1p[:64] = lf[b][128:]
        pk = np.concatenate(
            [
                xw.reshape(128, 2 * (N + H)),
                lf[b][:128],
                xb1p,
                watt,
                eye,
            ],
            axis=1,
        )
        in_maps.append(
            {
                "bin8": bin8,
                "mv28": mv28,
                "wbin8": wbin8,
                "pk": np.ascontiguousarray(pk).astype(ml_dtypes.bfloat16),
                "battc": battc,
            }
        )
    return in_maps


def run_full(inputs, trace=False):
    from concourse.bass_utils import run_bass_kernel_spmd

    if "nc" not in _CACHE:
        _CACHE["nc"] = _build()
    nc = _CACHE["nc"]
    in_maps = _prep_inputs(
        inputs["local_feats"],
        inputs["binary_feats"],
        inputs["W_apair"],
        inputs["W_bin"],
        inputs["b_bin"],
        inputs["w_att"],
        inputs["b_att"],
    )
    res = run_bass_kernel_spmd(nc, in_maps, list(range(B)), trace=trace)
    out = np.stack([np.asarray(res.results[c]["out"], np.float32) for c in range(B)])
    return out, res


def kernel(**inputs):
    out, _ = run_full(inputs, trace=False)
    return out


# revision 13
# speedup vs baseline: 1.0096x; 1.0072x over previous
"""Trainium2 Bass kernel for nn_Attention_18726057410699 (gnn_message_passing).

Math (per sample b):
  y        = local_feats[b] @ W_apair                       # [192, 256]
  binv     = binary_feats[b] @ W_bin + b_bin                # [128,128,256]
  z[i,j,k] = y[i,k] + y[j,k] + (binv[i,j,k] if i<128 and j<128 else 0)
  s[i,j]   = sigmoid( sum_k relu(z[i,j,k]) * w_att[k] + b_att )
  out[i,h] = sum_j s[i,j] * local_feats[b][j,h]

Sharding: data-parallel over batch B=8 -> 8 cores, one sample each.

Implementation notes:
 - binary_feats sent as fp8e4 flat [114, 16384] (112 channels + ones-row that
   folds b_bin + zero pad); the binv matmul runs in DoubleRow perf mode with a
   stride-0-broadcast second contraction half against a zero-padded stationary,
   so it costs 0.5 cycles/row while the DMA stays in the efficient flat layout.
 - y_i + y_j folded into PSUM by a second fp8 DoubleRow matmul whose moving
   operand is a host-sent identity+indicator pattern (per 4-row burst).
 - One relu pass moves PSUM->SBUF bf16 per [128,1024] supertile, split across
   Act and DVE (GPSIMD cannot access PSUM, so Pool is not eligible).
 - Attention scores via F=1 matmuls: stationary = relu'd z tile, moving =
   w_att column; accumulated into persistent PSUM score tiles (S^T layout).
 - Rows i>=128 (no binv term): fused add+relu from SBUF, early rows on DVE and
   late rows on Pool; symmetric block s[i<128, j>=128] filled by one PE
   transpose after an early mid-loop sigmoid.
 - Big DMAs are split across the SP/Act/Pool queues, whose transfers overlap.
"""

import numpy as np

B, N, H, L, C = 8, 192, 256, 128, 112
NIJ = L * L  # 16384

_CACHE = {}

ENGINE_SEM = {
    "EngineType.PE": "PE_",
    "EngineType.DVE": "DVE_",
    "EngineType.Activation": "Activation_",
    "EngineType.Pool": "Pool_",
    "EngineType.SP": "SP_",
}


def _fix_sync_waits(nc):
    """walrus accepts at most ONE sync-wait per compute instruction; Tile
    emits several.  Drop self-waits and push overflow onto earlier
    same-engine instructions (strictly more conservative)."""
    import dataclasses
    from collections import defaultdict

    il = [i for i in nc.all_instructions()]
    streams = defaultdict(list)
    for inst in il:
        si = getattr(inst, "sync_info", None)
        if si is None:
            continue
        upd = {u.ant_name for u in si.on_update}
        eng = str(getattr(inst, "engine", None))
        self_pfx = ENGINE_SEM.get(eng)
        keep = {}
        for w in si.on_wait:
            if w.ant_name in upd:
                continue
            if self_pfx and w.ant_name.startswith(self_pfx):
                continue
            k = w.ant_name
            if k not in keep or keep[k].wait_value < w.wait_value:
                keep[k] = w
        new = list(keep.values())
        if len(new) != len(si.on_wait):
            inst.sync_info = dataclasses.replace(si, on_wait=new)
        if type(inst).__name__ in (
            "InstMatmult", "InstTensorCopy", "InstTensorTensor",
            "InstTensorScalarPtr", "InstActivation", "InstMemset",
            "InstTensorReduce", "InstTensorTensorReduce",
        ):
            streams[eng].append(inst)

    for eng, insts in streams.items():
        overflow = []
        for inst in reversed(insts):
            si = inst.sync_info
            waits = list(si.on_wait) + overflow
            ded = {}
            for w in waits:
                if w.ant_name not in ded or ded[w.ant_name].wait_value < w.wait_value:
                    ded[w.ant_name] = w
            waits = list(ded.values())
            if len(waits) <= 1:
                inst.sync_info = dataclasses.replace(si, on_wait=waits)
                overflow = []
            else:
                inst.sync_info = dataclasses.replace(si, on_wait=[waits[-1]])
                overflow = waits[:-1]
        if overflow:
            raise RuntimeError(f"{eng}: could not place {len(overflow)} waits")


HI_SPLIT = 12  # region-B rows below this go to DVE, the rest to Pool

# relu-move engine schedule per supertile (32 supertiles = 32 bursts x 2kt):
# legal engines for PSUM reads: Act + DVE only
RELU_ENG = (
    "A A A D A A D A D A D A D A D A "
    "D A D A D A D A D A D A D A D A"
).split()


def _build():
    import concourse.bass as bass
    import concourse.tile as tile
    from concourse import bacc, mybir

    f32 = mybir.dt.float32
    bf16 = mybir.dt.bfloat16
    fp8 = mybir.dt.float8e4
    ALU = mybir.AluOpType
    ACTF = mybir.ActivationFunctionType
    PM = mybir.MatmulPerfMode

    nc = bacc.Bacc()

    p_bin = nc.declare_dram_parameter("bin8", [114, NIJ], fp8, isOutput=False)
    p_mv2 = nc.declare_dram_parameter("mv28", [128, 32, 512], fp8, isOutput=False)
    p_wbin = nc.declare_dram_parameter("wbin8", [114, 2, H], fp8, isOutput=False)
    # packed bf16 smalls: [xw (2*(N+H)) | xb (H) | xb1-pad (H) | watt (2) | eye (128)]
    PK = 2 * (N + H) + H + H + 2 + 128
    p_pk = nc.declare_dram_parameter("pk", [128, PK], bf16, isOutput=False)
    p_batt = nc.declare_dram_parameter("battc", [128, 1], f32, isOutput=False)
    p_out = nc.declare_dram_parameter("out", [N, H], f32, isOutput=True)

    with tile.TileContext(nc) as tc:
        with (
            tc.tile_pool(name="const", bufs=1) as cpool,
            tc.tile_pool(name="work", bufs=4) as wpool,
            tc.tile_pool(name="ub", bufs=66) as ubpool,
            tc.tile_pool(name="fin", bufs=1) as fpool,
            tc.tile_pool(name="pmain", bufs=3, space=bass.MemorySpace.PSUM) as pmain,
            tc.tile_pool(name="pscore", bufs=1, space=bass.MemorySpace.PSUM) as psc,
        ):
            # ---------------- DMA plan (flat 2D layouts; queues overlap) ----------------
            # SP: pk, battc, mv28 q3 q4, outs
            # Act-queue: bin8-h1 (bursts 0-15); Pool-queue: mv28 q1 q2, bin8-h2, wbin8
            bin8 = cpool.tile([114, NIJ], fp8, tag="bin8")
            mv28 = cpool.tile([128, 32, 512], fp8, tag="mv28")
            yn08 = cpool.tile([128, 2, H], fp8, tag="yn08")
            actwarm = cpool.tile([1, 2], bf16, tag="actwarm")
            nc.gpsimd.memset(actwarm[:, :], 0.0)
            nc.gpsimd.memset(yn08[:, 1, :], 0.0)
            # warm the sigmoid table at t=0 (its func-set also contains Relu)
            nc.scalar.activation(actwarm[:, 1:2], actwarm[:, 0:1], ACTF.Sigmoid, bias=0.0, scale=1.0)
            HLF = NIJ // 2
            QTR = 8
            pk = cpool.tile([128, PK], bf16, tag="pk")
            nc.sync.dma_start(pk[:, :], p_pk[:, :])
            wbin8 = cpool.tile([114, 2, H], fp8, tag="wbin8")
            nc.sync.dma_start(wbin8[:, :, :], p_wbin[:, :, :])
            bc_sb = cpool.tile([128, 1], f32, tag="battc")
            BQ = NIJ // 4
            BE = NIJ // 8
            nc.sync.dma_start(bin8[:, 0:BE], p_bin[:, 0:BE])
            nc.gpsimd.dma_start(mv28[:, 0:QTR, :], p_mv2[:, 0:QTR, :])
            nc.sync.dma_start(bin8[:, BE:BQ], p_bin[:, BE:BQ])
            nc.sync.dma_start(bin8[:, BQ : 2 * BQ], p_bin[:, BQ : 2 * BQ])
            nc.gpsimd.dma_start(mv28[:, QTR : 2 * QTR, :], p_mv2[:, QTR : 2 * QTR, :])
            nc.sync.dma_start(bin8[:, 2 * BQ : 3 * BQ], p_bin[:, 2 * BQ : 3 * BQ])
            nc.sync.dma_start(bin8[:, 3 * BQ : 4 * BQ], p_bin[:, 3 * BQ : 4 * BQ])
            nc.sync.dma_start(mv28[:, 2 * QTR : 3 * QTR, :], p_mv2[:, 2 * QTR : 3 * QTR, :])
            nc.sync.dma_start(mv28[:, 3 * QTR : 4 * QTR, :], p_mv2[:, 3 * QTR : 4 * QTR, :])
            nc.sync.dma_start(bc_sb[:, :], p_batt[:, :])

            o_xw = 0
            o_xb = 2 * (N + H)
            o_xb1 = o_xb + H
            o_wa = o_xb1 + H
            o_eye = o_wa + 2
            xw = pk[:, o_xw : o_xw + 2 * (N + H)].rearrange("p (t n) -> p t n", t=2)
            wa = pk[:, o_wa : o_wa + 2]
            xb = pk[:, o_xb : o_xb + H]
            xb1 = pk[0:64, o_xb1 : o_xb1 + H]
            eye_b = pk[:, o_eye : o_eye + 128]

            # ---------------- y matmuls (bf16) ----------------
            # Yn (fold stationary) first: it gates the first bursts
            ynp = pmain.tile([128, 2, 512], f32, tag="super")
            for ht in range(2):
                nc.tensor.matmul(
                    ynp[:, 0, 0:H],
                    xw[:, ht, 0:128],
                    xw[:, ht, N : N + H],
                    start=(ht == 0),
                    stop=(ht == 1),
                )
            nc.scalar.copy(yn08[:, 0, :], ynp[:, 0, 0:H])

            ytp = []
            for kt in range(2):
                ps = pmain.tile([128, 2, 512], f32, tag="super")
                for ht in range(2):
                    nc.tensor.matmul(
                        ps[:, 0, 0:N],
                        xw[:, ht, N + kt * 128 : N + (kt + 1) * 128],
                        xw[:, ht, 0:N],
                        start=(ht == 0),
                        stop=(ht == 1),
                    )
                ytp.append(ps)
            ytb = cpool.tile([128, 2, N], bf16, tag="ytb")
            yt_f = cpool.tile([128, 2, N], f32, tag="ytf")
            for kt in range(2):
                nc.vector.tensor_copy(ytb[:, kt, :], ytp[kt][:, 0, 0:N])
                nc.vector.tensor_copy(yt_f[:, kt, :], ytp[kt][:, 0, 0:N])

            # ---------------- score PSUM tiles (persistent) ----------------
            stt = psc.tile([128, 2, N], f32, tag="st")
            st0 = stt[:, 0, :]                            # S^T[j<128, i]
            st1 = stt[0:64, 1, :]                         # S^T[j>=128, i] (cols i>=128 only)

            # ---------------- main interleaved loop ----------------
            hi_tiles = {}
            lo_tiles = {}

            def hi_compute(t):
                # region B: i = 128 + t; early rows on DVE, late rows on Pool
                # (GPSIMD cannot touch PSUM, so Pool earns its keep here)
                i = 128 + t
                u = ubpool.tile([128, 2, N], bf16, tag="ub")
                eng = nc.vector if t < HI_SPLIT else nc.gpsimd
                for kt in range(2):
                    eng.tensor_scalar(
                        u[:, kt, :], ytb[:, kt, :],
                        yt_f[:, kt, i : i + 1], 0.0,
                        ALU.add, ALU.max,
                    )
                hi_tiles[t] = u

            def hi_scores(t):
                i = 128 + t
                u = hi_tiles.pop(t)
                for kt in range(2):
                    nc.tensor.matmul(
                        stt[:, 0, i : i + 1], u[:, kt, 0:128], wa[:, kt : kt + 1],
                        start=(kt == 0), stop=(kt == 1), skip_group_check=True,
                    )
                for kt in range(2):
                    nc.tensor.matmul(
                        stt[0:64, 1, i : i + 1], u[:, kt, 128:N], wa[:, kt : kt + 1],
                        start=(kt == 0), stop=(kt == 1), skip_group_check=True,
                    )

            def lo_burst(ib):
                # region A: i = 4*ib..4*ib+3, j<128; z for both kt in one supertile
                ps = pmain.tile([128, 2, 512], f32, tag="super")
                rhs_bin = bin8[:, ib * 512 : (ib + 1) * 512].unsqueeze(1).broadcast_to([114, 2, 512])
                rhs_mv = mv28[:, ib, :].unsqueeze(1).broadcast_to([128, 2, 512])
                for kt in range(2):
                    nc.tensor.matmul(
                        ps[:, kt, :],
                        wbin8[:, :, kt * 128 : (kt + 1) * 128],
                        rhs_bin,
                        start=True, stop=False,
                        perf_mode=PM.DoubleRow, skip_group_check=True,
                    )
                    nc.tensor.matmul(
                        ps[:, kt, :],
                        yn08[:, :, kt * 128 : (kt + 1) * 128],
                        rhs_mv,
                        start=False, stop=True,
                        perf_mode=PM.DoubleRow, skip_group_check=True,
                    )
                u = wpool.tile([128, 2, 4, 128], bf16, tag="u")
                if RELU_ENG[ib] == "A":
                    nc.scalar.activation(u[:, :, :, :], ps[:, :, :], ACTF.Relu, bias=0.0, scale=1.0)
                else:
                    nc.vector.tensor_scalar(u[:, :, :, :], ps[:, :, :], 0.0, None, ALU.max)
                lo_tiles[ib] = u

            def lo_scores(ib):
                u = lo_tiles.pop(ib)
                for m in range(4):
                    i = 4 * ib + m
                    for kt in range(2):
                        nc.tensor.matmul(
                            stt[:, 0, i : i + 1], u[:, kt, m, :], wa[:, kt : kt + 1],
                            start=(kt == 0), stop=(kt == 1), skip_group_check=True,
                        )

            # all region-B DVE work upfront (independent of binT DMA);
            # their cheap F=1 score matmuls interleave between bursts
            for t in range(64):
                hi_compute(t)
            hi_next = 0
            ss0 = fpool.tile([128, N], bf16, tag="ss0")
            ss1 = fpool.tile([64, N], bf16, tag="ss1")
            for ib in range(32):
                lo_burst(ib)
                if ib >= 2:
                    lo_scores(ib - 2)
                nhi = 2 if ib < 20 else 3
                for _ in range(nhi):
                    if hi_next < 64:
                        hi_scores(hi_next)
                        hi_next += 1
                if ib == 30:
                    # B-columns complete: sigmoid them, mirror, and do the
                    # i>=128 half of the finale while region A still runs
                    nc.scalar.activation(ss0[:, 128:N], stt[:, 0, 128:N], ACTF.Sigmoid,
                                         bias=bc_sb[:, 0:1], scale=1.0)
                    nc.scalar.activation(ss1[:, 128:N], stt[0:64, 1, 128:N], ACTF.Sigmoid,
                                         bias=bc_sb[0:64, 0:1], scale=1.0)
                    pst = psc.tile([64, 128], bf16, tag="trsp")
                    nc.tensor.transpose(pst[:, :], ss0[:, 128:N], eye_b)
                    nc.vector.tensor_copy(ss1[:, 0:128], pst[:, :])

            lo_scores(30)
            lo_scores(31)

            # ---------------- finale ----------------
            # i>=128 half first (B-side sigmoids already done mid-loop)
            po1 = pmain.tile([128, 2, 512], f32, tag="super")
            nc.tensor.matmul(po1[0:64, 0, 0:H], ss0[:, 128:N], xb, start=True, stop=False)
            nc.tensor.matmul(po1[0:64, 0, 0:H], ss1[:, 128:N], xb1, start=False, stop=True)
            nc.scalar.activation(ss0[:, 0:128], stt[:, 0, 0:128], ACTF.Sigmoid,
                                 bias=bc_sb[:, 0:1], scale=1.0)
            ob1 = fpool.tile([64, H], f32, tag="ob1")
            nc.vector.tensor_copy(ob1[:, :], po1[0:64, 0, 0:H])
            nc.gpsimd.dma_start(p_out[128:N, :], ob1[:, :])
            po = pmain.tile([128, 2, 512], f32, tag="super")
            nc.tensor.matmul(po[:, 0, 0:H], ss1[:, 0:128], xb1, start=True, stop=False)
            nc.tensor.matmul(po[:, 0, 0:H], ss0[:, 0:128], xb, start=False, stop=True)
            ob = fpool.tile([128, H], f32, tag="ob0")
            nc.vector.tensor_copy(ob[:, :], po[:, 0, 0:H])
            nc.sync.dma_start(p_out[0:128, :], ob[:, :])

    nc.compile()
    return nc


def _mv2_const():
    import ml_dtypes
    # mv2[p, ib, m*128+j] = I[p == j] + I[p == 4*ib+m]
    mv = np.zeros((128, 32, 4, 128), np.float32)
    for p in range(128):
        mv[p, :, :, p] += 1.0
        ib, m = divmod(p, 4)
        mv[p, ib, m, :] += 1.0
    return np.ascontiguousarray(mv.reshape(128, 32, 512)).astype(ml_dtypes.float8_e4m3)


def _prep_inputs(local_feats, binary_feats, W_apair, W_bin, b_bin, w_att, b_att):
    import ml_dtypes
    fp8 = ml_dtypes.float8_e4m3

    lf = np.asarray(local_feats, np.float32)
    bf = np.asarray(binary_feats, np.float32)
    wap = np.ascontiguousarray(np.asarray(W_apair, np.float32))
    wbin = np.asarray(W_bin, np.float32)
    bb = np.asarray(b_bin, np.float32).reshape(H)
    wa = np.asarray(w_att, np.float32).reshape(H)
    battc = np.full((128, 1), np.float32(np.asarray(b_att).reshape(-1)[0]), np.float32)

    # wbin8[p, d, k]: d0 = wbin_aug row p (c=112 -> b_bin, c=113 -> 0), d1 = zeros
    wbin_aug = np.zeros((114, 2, H), np.float32)
    wbin_aug[:C, 0] = wbin
    wbin_aug[C, 0] = bb
    wbin8 = np.ascontiguousarray(wbin_aug).astype(fp8)

    mv28 = _CACHE.get("mv28")
    if mv28 is None:
        mv28 = _CACHE["mv28"] = _mv2_const()
    watt = np.ascontiguousarray(wa.reshape(2, 128).T)
    eye = np.eye(128, dtype=np.float32)

    in_maps = []
    for b in range(B):
        # bin8[c, ij]: rows 0..111 = binT, 112 = ones (b_bin), 113 = zeros
        binT = bf[b].reshape(NIJ, C).T
        bin_aug = np.empty((114, NIJ), np.float32)
        bin_aug[:C] = binT
        bin_aug[C] = 1.0
        bin_aug[C + 1] = 0.0
        bin8 = np.ascontiguousarray(bin_aug).astype(fp8)

        # xw[p, ht, :]: cols 0..191 = x^T[h, j] (h = ht*128+p), cols 192.. = W_apair[h, k]
        xT = lf[b].T  # [256, 192]
        xw = np.concatenate(
            [xT.reshape(2, 128, N), wap.reshape(2, 128, H)], axis=2
        ).transpose(1, 0, 2)  # [128, 2, N+H]
        xb1p = np.zeros((128, H), np.float32)
        xb1p[:64] = lf[b][128:]
        pk = np.concatenate(
            [
                xw.reshape(128, 2 * (N + H)),
                lf[b][:128],
                xb1p,
                watt,
                eye,
            ],
            axis=1,
        )
        in_maps.append(
            {
                "bin8": bin8,
                "mv28": mv28,
                "wbin8": wbin8,
                "pk": np.ascontiguousarray(pk).astype(ml_dtypes.bfloat16),
                "battc": battc,
            }
        )
    return in_maps


def run_full(inputs, trace=False):
    from concourse.bass_utils import run_bass_kernel_spmd

    if "nc" not in _CACHE:
        _CACHE["nc"] = _build()
    nc = _CACHE["nc"]
    in_maps = _prep_inputs(
        inputs["local_feats"],
        inputs["binary_feats"],
        inputs["W_apair"],
        inputs["W_bin"],
        inputs["b_bin"],
        inputs["w_att"],
        inputs["b_att"],
    )
    res = run_bass_kernel_spmd(nc, in_maps, list(range(B)), trace=trace)
    out = np.stack([np.asarray(res.results[c]["out"], np.float32) for c in range(B)])
    return out, res


def kernel(**inputs):
    out, _ = run_full(inputs, trace=False)
    return out
